# revision 1
# baseline (speedup 1.0000x reference)
"""Trainium2 Bass kernel for the spectral ConvolutionLayer problem.

Math: with u (B=2, L=4096, D=256), eigenvectors ev (K=16, L), eigenvalues
lam (K,), M (K, 256, 256):

    conv[b,k,d,l] = sum_t u[b,t,d] * ev[k, (l-t) mod L]       (circular conv)
    out[b,l,p]    = sum_{k,d} conv[b,k,d,l] * lam[k] * M[k,d,p]

Equivalently out[b] = sum_k (C_k @ u[b]) @ (lam_k M_k) with the circulant
C_k[l,t] = ev[k,(l-t) mod L].

Sharding: output rows l are sharded across 8 cores (512 rows each) — no
collectives.  Per core and filter k the needed circulant slice is a Hankel
matrix in disguise: after reversing the partition order of both matmul
operands, its tiles become plain overlapping-window reads
C_buf[q, col] = ev_ext[q + col] from a small host-prepared extended buffer
ev_ext[i] = ev[k, (l_off + i - (L-1)) mod L].  One 1.1 MB DMA per filter
materializes all circulant tiles for that filter.

Per-core pipeline (all matmuls bf16, fp32 PSUM accumulate):
  stage 1: y[b,k][d,l'] = sum_jr u_rev_tile(jr)^T @ C_buf[:, 128jr:128jr+512]
  stage 2: outT[b][p,l'] += (lam_k M_k)[d,p]^T-side matmul with rhs y
Output is written transposed (B, D, 512) and fixed up on host.
"""

import numpy as np
import ml_dtypes

import concourse.bacc as bacc
import concourse.bass as bass
import concourse.mybir as mybir
import concourse.tile as tile
from concourse.bass_utils import run_bass_kernel_spmd

B, L, D, K = 2, 4096, 256, 16
NCORES = 8
LSH = L // NCORES          # 512 output rows per core
NJR = L // 128             # 32 contraction tiles
EXT = 4608                 # extended eigenvector buffer length (>= 128*31+512+127+1)
CB_W = 128 * (NJR - 1) + LSH   # 4480 C-buffer width
NREP_EV = 8                # HBM replicas of ev_ext to spread DMA hot-spot
BF16 = mybir.dt.bfloat16
F32 = mybir.dt.float32
NPBF16 = ml_dtypes.bfloat16

_CACHE = {}


def _build_bass():
    nc = bacc.Bacc("TRN2", target_bir_lowering=False)
    u_h = nc.dram_tensor("u_rev", [B, L, D], BF16, kind="ExternalInput")
    m_h = nc.dram_tensor("m_mat", [K, 2, 128, D], BF16, kind="ExternalInput")
    # 8 identical replicas of the extended eigenvector buffer.  The C-buffer
    # expansion reads ~18 MB through overlapping windows over a ~9 KB
    # footprint per filter; replicas spread concurrent SDMA reads across 8x
    # more HBM pages to avoid bank hot-spotting.
    e_h = nc.dram_tensor("ev_ext", [NREP_EV, K, EXT], BF16, kind="ExternalInput")
    o_h = nc.dram_tensor("out_t", [B, D, LSH], F32, kind="ExternalOutput")

    with tile.TileContext(nc) as tc:
        with (
            tc.tile_pool(name="const", bufs=1) as const_pool,
            tc.tile_pool(name="cbufp", bufs=2) as cbuf_pool,
            tc.tile_pool(name="ysbp", bufs=8) as y_pool,
            tc.tile_pool(name="osbp", bufs=2) as o_pool,
            tc.tile_pool(name="psyp", bufs=2, space="PSUM") as psy_pool,
            tc.tile_pool(name="psop", bufs=4, space="PSUM") as pso_pool,
        ):
            # u_sb[ti, b, jr, d] = u_rev[b, 128*jr + ti, d]
            u_sb = const_pool.tile([128, B, NJR, D], BF16, name="u_sb")
            nc.sync.dma_start(
                out=u_sb, in_=u_h[:].rearrange("b (jr p) d -> p b jr d", p=128)
            )
            # m_sb[di, k, dh, p] = m_mat[k, dh, di, p]  (= lam_k*M[k, 128dh+di, p])
            m_sb = const_pool.tile([128, K, 2, D], BF16, name="m_sb")
            nc.sync.dma_start(out=m_sb, in_=m_h[:].rearrange("k h i p -> i k h p"))

            # output accumulators: outT[b][ph][p_i, l'] , p = 128*ph + p_i
            out_ps = {}
            for b in range(B):
                for ph in range(2):
                    out_ps[(b, ph)] = pso_pool.tile(
                        [128, LSH], F32, name=f"out_ps_{b}_{ph}",
                        tag=f"out_ps_{b}_{ph}", bufs=1,
                    )

            for k in range(K):
                # C_buf[q, col] = ev_ext[k, q + col] : overlapping-window DMA
                # C_buf[q, col] = ev_ext[k, q + col] : overlapping-window DMAs,
                # one 16-partition group per eigenvector replica.
                cb = cbuf_pool.tile([128, CB_W], BF16, name="cb", tag="cb")
                rows = 128 // NREP_EV
                for i in range(NREP_EV):
                    nc.gpsimd.dma_start(
                        out=cb[rows * i:rows * (i + 1), :],
                        in_=bass.AP(
                            e_h,
                            (i * K + k) * EXT + rows * i,
                            [[1, rows], [1, CB_W]],
                        ),
                    )
                for b in range(B):
                    for dh in range(2):
                        psy = psy_pool.tile([128, LSH], F32, name="psy", tag="psy")
                        for jr in range(NJR):
                            nc.tensor.matmul(
                                psy,
                                u_sb[:, b, jr, dh * 128:(dh + 1) * 128],
                                cb[:, 128 * jr:128 * jr + LSH],
                                start=(jr == 0),
                                stop=(jr == NJR - 1),
                            )
                        ysb = y_pool.tile([128, LSH], BF16, name="ysb", tag="ysb")
                        nc.vector.tensor_copy(ysb, psy)
                        for ph in range(2):
                            nc.tensor.matmul(
                                out_ps[(b, ph)],
                                m_sb[:, k, dh, ph * 128:(ph + 1) * 128],
                                ysb,
                                start=(k == 0 and dh == 0),
                                stop=(k == K - 1 and dh == 1),
                            )

            for b in range(B):
                for ph in range(2):
                    osb = o_pool.tile([128, LSH], F32, name="osb", tag="osb")
                    nc.vector.tensor_copy(osb, out_ps[(b, ph)])
                    nc.sync.dma_start(
                        out=o_h[b, ph * 128:(ph + 1) * 128, :], in_=osb
                    )
    nc.finalize()
    return nc


def _prep_inputs(u, eigenvectors, eigenvalues, M):
    u = np.asarray(u, dtype=np.float32)
    ev = np.asarray(eigenvectors, dtype=np.float32)
    lam = np.asarray(eigenvalues, dtype=np.float32)
    M = np.asarray(M, dtype=np.float32)

    u_rev = np.ascontiguousarray(u[:, ::-1, :]).astype(NPBF16)
    m_mat = np.ascontiguousarray(
        (lam[:, None, None] * M).astype(NPBF16).reshape(K, 2, 128, D)
    )
    in_maps = []
    idx = np.arange(EXT)
    for c in range(NCORES):
        l_off = LSH * c
        ev_ext = ev[:, (l_off + idx - (L - 1)) % L].astype(NPBF16)
        ev_rep = np.ascontiguousarray(
            np.broadcast_to(ev_ext[None], (NREP_EV, K, EXT))
        )
        in_maps.append({"u_rev": u_rev, "m_mat": m_mat, "ev_ext": ev_rep})
    return in_maps


def _run(inputs, trace=False):
    if "nc" not in _CACHE:
        _CACHE["nc"] = _build_bass()
    nc = _CACHE["nc"]
    in_maps = _prep_inputs(**inputs)
    res = run_bass_kernel_spmd(
        nc, in_maps, core_ids=list(range(NCORES)), trace=trace
    )
    out = np.empty((B, L, D), dtype=np.float32)
    for c in range(NCORES):
        out[:, LSH * c:LSH * (c + 1), :] = np.asarray(
            res.results[c]["out_t"]
        ).transpose(0, 2, 1)
    return out, res


def kernel(**inputs):
    out, _ = _run(inputs, trace=False)
    return out



# revision 4
# speedup vs baseline: 1.0043x; 1.0043x over previous
"""Trainium2 Bass kernel for the spectral ConvolutionLayer problem.

Math: with u (B=2, L=4096, D=256), eigenvectors ev (K=16, L), eigenvalues
lam (K,), M (K, 256, 256):

    conv[b,k,d,l] = sum_t u[b,t,d] * ev[k, (l-t) mod L]       (circular conv)
    out[b,l,p]    = sum_{k,d} conv[b,k,d,l] * lam[k] * M[k,d,p]

Equivalently out[b] = sum_k (C_k @ u[b]) @ (lam_k M_k) with the circulant
C_k[l,t] = ev[k,(l-t) mod L].

Sharding: output rows l are sharded across 8 cores (512 rows each) — no
collectives.  Per core and filter k the needed circulant slice is a Hankel
matrix in disguise: after reversing the partition order of both matmul
operands, its tiles become plain overlapping-window reads
C_buf[q, col] = ev_ext[q + col] from a small host-prepared extended buffer
ev_ext[i] = ev[k, (l_off + i - (L-1)) mod L].  One 1.1 MB DMA per filter
materializes all circulant tiles for that filter.

Per-core pipeline (all matmuls bf16, fp32 PSUM accumulate):
  stage 1: y[b,k][d,l'] = sum_jr u_rev_tile(jr)^T @ C_buf[:, 128jr:128jr+512]
  stage 2: outT[b][p,l'] += (lam_k M_k)[d,p]^T-side matmul with rhs y
Output is written transposed (B, D, 512) and fixed up on host.
"""

import numpy as np
import ml_dtypes

import concourse.bacc as bacc
import concourse.bass as bass
import concourse.mybir as mybir
import concourse.tile as tile
from concourse.bass_utils import run_bass_kernel_spmd

B, L, D, K = 2, 4096, 256, 16
NCORES = 8
LSH = L // NCORES          # 512 output rows per core
NJR = L // 128             # 32 contraction tiles
EXT = 4608                 # extended eigenvector buffer length (>= 128*31+512+127+1)
CB_W = 128 * (NJR - 1) + LSH   # 4480 C-buffer width
NREP_EV = 8                # HBM replicas of ev_ext to spread DMA hot-spot
BF16 = mybir.dt.bfloat16
F32 = mybir.dt.float32
NPBF16 = ml_dtypes.bfloat16

_CACHE = {}


def _build_bass():
    nc = bacc.Bacc("TRN2", target_bir_lowering=False)
    u_h = nc.dram_tensor("u_rev", [B, L, D], BF16, kind="ExternalInput")
    m_h = nc.dram_tensor("m_mat", [K, 2, 128, D], BF16, kind="ExternalInput")
    # 8 identical replicas of the extended eigenvector buffer.  The C-buffer
    # expansion reads ~18 MB through overlapping windows over a ~9 KB
    # footprint per filter; replicas spread concurrent SDMA reads across 8x
    # more HBM pages to avoid bank hot-spotting.
    e_h = nc.dram_tensor("ev_ext", [NREP_EV, K, EXT], BF16, kind="ExternalInput")
    o_h = nc.dram_tensor("out_t", [B, D, LSH], F32, kind="ExternalOutput")

    with tile.TileContext(nc) as tc:
        with (
            tc.tile_pool(name="const", bufs=1) as const_pool,
            tc.tile_pool(name="cbufp", bufs=2) as cbuf_pool,
            tc.tile_pool(name="ysbp", bufs=8) as y_pool,
            tc.tile_pool(name="osbp", bufs=2) as o_pool,
            tc.tile_pool(name="psyp", bufs=2, space="PSUM") as psy_pool,
            tc.tile_pool(name="psop", bufs=4, space="PSUM") as pso_pool,
        ):
            # m_sb[di, k, dh, p] = m_mat[k, dh, di, p]  (= lam_k*M[k, 128dh+di, p])
            m_sb = const_pool.tile([128, K, 2, D], BF16, name="m_sb")
            nc.sync.dma_start(out=m_sb, in_=m_h[:].rearrange("k h i p -> i k h p"))
            # u_sb[ti, b, jr, d] = u_rev[b, 128*jr + ti, d].  Split into 8
            # slice DMAs (issued in consumption order) so the first stage-1
            # matmul group only waits on its 528 KB slice, not the full
            # 4.2 MB; the rest streams in under compute.
            u_sb = const_pool.tile([128, B, NJR, D], BF16, name="u_sb")
            JQ = NJR // 4
            for b in range(B):
                for jq in range(4):
                    nc.sync.dma_start(
                        out=u_sb[:, b, JQ * jq:JQ * (jq + 1), :],
                        in_=u_h[b, 128 * JQ * jq:128 * JQ * (jq + 1), :]
                        .rearrange("(jr p) d -> p jr d", p=128),
                    )

            # Warm the PE pstate ramp while the u/cb DMAs are in flight:
            # dummy matmuls on m_sb (tiny DMA, lands first) keep the PE busy
            # so the main loop enters at full clock instead of ramping.
            with tc.tile_pool(name="warmp", bufs=1, space="PSUM") as warm_pool:
                warm_ps = warm_pool.tile([128, 2 * D], F32, name="warm_ps")
                for w in range(5):
                    nc.tensor.matmul(
                        warm_ps,
                        m_sb[:, 0, 0, 0:128],
                        m_sb[:, w, :, :],
                        start=(w == 0),
                        stop=(w == 4),
                    )

            # output accumulators: outT[b][ph][p_i, l'] , p = 128*ph + p_i
            out_ps = {}
            for b in range(B):
                for ph in range(2):
                    out_ps[(b, ph)] = pso_pool.tile(
                        [128, LSH], F32, name=f"out_ps_{b}_{ph}",
                        tag=f"out_ps_{b}_{ph}", bufs=1,
                    )

            for k in range(K):
                # C_buf[q, col] = ev_ext[k, q + col] : overlapping-window DMA
                # C_buf[q, col] = ev_ext[k, q + col] : overlapping-window DMAs,
                # one 16-partition group per eigenvector replica.
                cb = cbuf_pool.tile([128, CB_W], BF16, name="cb", tag="cb")
                rows = 128 // NREP_EV
                for i in range(NREP_EV):
                    nc.gpsimd.dma_start(
                        out=cb[rows * i:rows * (i + 1), :],
                        in_=bass.AP(
                            e_h,
                            (i * K + k) * EXT + rows * i,
                            [[1, rows], [1, CB_W]],
                        ),
                    )
                for b in range(B):
                    for dh in range(2):
                        psy = psy_pool.tile([128, LSH], F32, name="psy", tag="psy")
                        for jr in range(NJR):
                            nc.tensor.matmul(
                                psy,
                                u_sb[:, b, jr, dh * 128:(dh + 1) * 128],
                                cb[:, 128 * jr:128 * jr + LSH],
                                start=(jr == 0),
                                stop=(jr == NJR - 1),
                            )
                        ysb = y_pool.tile([128, LSH], BF16, name="ysb", tag="ysb")
                        nc.vector.tensor_copy(ysb, psy)
                        for ph in range(2):
                            nc.tensor.matmul(
                                out_ps[(b, ph)],
                                m_sb[:, k, dh, ph * 128:(ph + 1) * 128],
                                ysb,
                                start=(k == 0 and dh == 0),
                                stop=(k == K - 1 and dh == 1),
                            )

            for b in range(B):
                for ph in range(2):
                    osb = o_pool.tile([128, LSH], F32, name="osb", tag="osb")
                    nc.vector.tensor_copy(osb, out_ps[(b, ph)])
                    nc.sync.dma_start(
                        out=o_h[b, ph * 128:(ph + 1) * 128, :], in_=osb
                    )
    nc.finalize()
    return nc


def _prep_inputs(u, eigenvectors, eigenvalues, M):
    u = np.asarray(u, dtype=np.float32)
    ev = np.asarray(eigenvectors, dtype=np.float32)
    lam = np.asarray(eigenvalues, dtype=np.float32)
    M = np.asarray(M, dtype=np.float32)

    u_rev = np.ascontiguousarray(u[:, ::-1, :]).astype(NPBF16)
    m_mat = np.ascontiguousarray(
        (lam[:, None, None] * M).astype(NPBF16).reshape(K, 2, 128, D)
    )
    in_maps = []
    idx = np.arange(EXT)
    for c in range(NCORES):
        l_off = LSH * c
        ev_ext = ev[:, (l_off + idx - (L - 1)) % L].astype(NPBF16)
        ev_rep = np.ascontiguousarray(
            np.broadcast_to(ev_ext[None], (NREP_EV, K, EXT))
        )
        in_maps.append({"u_rev": u_rev, "m_mat": m_mat, "ev_ext": ev_rep})
    return in_maps


def _run(inputs, trace=False):
    if "nc" not in _CACHE:
        _CACHE["nc"] = _build_bass()
    nc = _CACHE["nc"]
    in_maps = _prep_inputs(**inputs)
    res = run_bass_kernel_spmd(
        nc, in_maps, core_ids=list(range(NCORES)), trace=trace
    )
    out = np.empty((B, L, D), dtype=np.float32)
    for c in range(NCORES):
        out[:, LSH * c:LSH * (c + 1), :] = np.asarray(
            res.results[c]["out_t"]
        ).transpose(0, 2, 1)
    return out, res


def kernel(**inputs):
    out, _ = _run(inputs, trace=False)
    return out

